# revision 22
# baseline (speedup 1.0000x reference)
"""PiLoraLayer TRN2 kernel: y = x + (alpha/r) * sin((2/pi) * (x @ A) @ B).

x: [4, 4096, 4096] f32; A = A_int8 * scale_A (per-col), B = B_int8 * scale_B
(per-col); rank 16 bottleneck.  alpha/r = 2.

Strategy v2 (data-parallel over 8 NeuronCores, TRANSPOSED compute space):

The v1 kernel was PE-bound: mm1 (h1 = x @ A) contracts over hidden, which
needs hidden on the partition axis, so every x tile went through a PE
transpose (512 fp32 transposes/core ~ 140us, plus 33% throttle).  v2 kills
all device transposes by shipping each core its token-shard PRE-TRANSPOSED
(hidden-major) from the host, and doing everything (mm1, mm2, sin, residual
add, output) in transposed space.

Precision plan (validated vs reference in numpy: rel err 6.1e-3 < 2e-2):
- Host ships xh = fp16(x.T / 2): fp16 halves DMA-in; /2 folds the final *2.
- mm1: h1[16, tok] = (2*A_int8 in fp16, exact).T @ xh  -> PSUM f32.
  (scale_A is folded into Bpn, so mm1 is exact except fp16(x).)
- mm2: fp32 matmuls run 4-pass on the PE (4x columns), so u is computed as
  ONE 1-pass bf16 matmul with the hi/lo split STACKED on the contraction
  axis (PE time ~ output columns, independent of K):
    u[128h, tok] = [h1_hi; h1_lo; h1_hi] (K=48) @ [Bp_hi; Bp_hi; Bp_lo]
  which is h1 @ Bpn to ~2^-17 relative.  Bpn = -scale_A x B x scale_B
  / pi^2, so 2*pi*u = -(2/pi)*h.  Bp hi/lo are split on the host; h1 hi/lo
  on device (2 ACT copies + 1 DVE subtract on a tiny [16, tok] tile).
- Range reduction (HW Sin LUT domain is [-pi, pi]; DVE `mod` fails the
  compiler ISA check, so): k = (u + 1.5*2^23) - 1.5*2^23 in one two-op DVE
  tensor_scalar (RNE round-to-int, bf16 exact for |k| <= 256); PE
  accumulates -k into the u PSUM bank via a bf16 negative-identity matmul,
  leaving frac in [-0.5, 0.5].
- ACT: s = fp16(Sin(frac * 2pi)) = +sin((2/pi) h).
- The final add rides the DMA engines' inline CCE ALU (gpsimd SWDGE
  accum_op=add): an SBUF->SBUF accum-DMA does x_sb += s, then a plain DMA
  streams x_sb (= y/2) out as fp16; host computes y = 2 * out.T.  This
  keeps the DVE down to ONE op per element (the MAGIC round).

Per-core budget (tokens=2048, hidden=4096): DMA 16 MiB in + 16 MiB out
(~91us floor at 358 GB/s); PE ~70-90us (small matmuls only); DVE 2 ops/elem
(~85us); ACT 1 op/elem (~60us).  Tokens are processed in 2 halves of 1024 so
the half-B input DMA overlaps the half-A tail.
"""

import sys

sys.path.insert(0, "/opt/trn_rl_repo")

import numpy as np

import concourse.bacc as bacc
import concourse.tile as tile
from concourse import mybir
from concourse.bass_utils import run_bass_kernel_spmd

P = 128
HIDDEN = 4096
RANK = 16
N_CORES = 8
TOTAL_ROWS = 4 * 4096
ROWS = TOTAL_ROWS // N_CORES  # 2048 tokens per core
KC = HIDDEN // P  # 32 hidden chunks
MAGIC = 12582912.0  # 1.5 * 2^23: f32 add/sub rounds to nearest integer
SCALE_2PI = 6.283185  # slightly < 2*pi: LUT arg stays inside [-pi, pi]

F32 = mybir.dt.float32
F32R = mybir.dt.float32r  # replicated fp32: 1 cycle/row on PE when N>=256
FP16 = mybir.dt.float16
BF16 = mybir.dt.bfloat16


def build_nc(tok: int = ROWS, split: int = 2):
    """Per-core program for a transposed [4096, tok] fp16 token shard."""
    halves = [tok // split] * split
    assert sum(halves) == tok and all(h % 512 == 0 for h in halves)

    nc = bacc.Bacc(
        "TRN2",
        target_bir_lowering=False,
        debug=False,
        enable_asserts=False,
        num_devices=N_CORES,
    )
    x_d = nc.dram_tensor("xh", [HIDDEN, tok], FP16, kind="ExternalInput").ap()
    a_d = nc.dram_tensor("A2", [HIDDEN, RANK], FP16, kind="ExternalInput").ap()
    bp_d = nc.dram_tensor("Bps", [6 * RANK, HIDDEN], BF16, kind="ExternalInput").ap()
    y_d = nc.dram_tensor("out", [HIDDEN, tok], FP16, kind="ExternalOutput").ap()

    with tile.TileContext(nc) as tc:
        with (
            tc.tile_pool(name="singles", bufs=1) as singles,
            tc.tile_pool(name="xp", bufs=2) as xpool,
            tc.tile_pool(name="h1p", bufs=1, space="PSUM") as h1_psum,
            tc.tile_pool(name="h1s", bufs=2) as h1pool,
            tc.tile_pool(name="up", bufs=3, space="PSUM") as u_psum,
            tc.tile_pool(name="tp", bufs=3) as tpool,
            tc.tile_pool(name="sp", bufs=3) as spool,
        ):
            a_sb = singles.tile([P, KC, RANK], FP16)
            nc.sync.dma_start(
                out=a_sb[:], in_=a_d.rearrange("(k p) r -> p k r", p=P)
            )
            bp_sb = singles.tile([6 * RANK, HIDDEN], BF16)
            nc.sync.dma_start(out=bp_sb[:], in_=bp_d[:, :])
            nident_bf = singles.tile([P, P], BF16)
            nc.gpsimd.memset(nident_bf[:], 0.0)
            nc.gpsimd.affine_select(
                out=nident_bf[:],
                in_=nident_bf[:],
                compare_op=mybir.AluOpType.not_equal,
                fill=-1.0,
                base=0,
                pattern=[[-1, P]],
                channel_multiplier=1,
            )

            t0 = 0
            for t_half in halves:
                nb = t_half // 512  # psum-bank-wide blocks in this half
                # ---- head: DMA x k-chunks in, mm1 accumulate over k ----
                x_sb = xpool.tile([P, KC, t_half], FP16)
                for k in range(KC):
                    nc.sync.dma_start(
                        out=x_sb[:, k, :],
                        in_=x_d[k * P : (k + 1) * P, t0 : t0 + t_half],
                    )
                h1_ps = h1_psum.tile([RANK, t_half], F32)
                for k in range(KC):
                    for b in range(nb):
                        nc.tensor.matmul(
                            h1_ps[:, b * 512 : (b + 1) * 512],
                            a_sb[:, k, :],
                            x_sb[:, k, b * 512 : (b + 1) * 512],
                            start=(k == 0),
                            stop=(k == KC - 1),
                        )
                # h1 hi/lo bf16 split, stacked [h1_hi; _; h1_lo; _; h1_hi; _]
                # at 32-aligned starts, for the single K=96 bf16 mm2
                h1_sb = h1pool.tile([6 * RANK, t_half], BF16)
                nc.gpsimd.memset(h1_sb[:], 0.0)
                nc.scalar.copy(out=h1_sb[0:RANK, :], in_=h1_ps[:])
                nc.vector.tensor_tensor(
                    h1_sb[2 * RANK : 3 * RANK, :],
                    h1_ps[:],
                    h1_sb[0:RANK, :],
                    mybir.AluOpType.subtract,
                )
                nc.scalar.copy(out=h1_sb[4 * RANK : 5 * RANK, :], in_=h1_ps[:])

                # ---- tail: chunk PAIRS: mm2+round+sin per chunk, then one
                # [128, 2*t_half] subtract and one two-row-group DMA out ----
                for cp in range(KC // 2):
                    s_sb = spool.tile([P, 2, t_half], FP16)
                    for ci in range(2):
                        c = 2 * cp + ci
                        u_ps = u_psum.tile([P, t_half], F32)
                        for b in range(nb):
                            nc.tensor.matmul(
                                u_ps[:, b * 512 : (b + 1) * 512],
                                bp_sb[:, c * P : (c + 1) * P],
                                h1_sb[:, b * 512 : (b + 1) * 512],
                                start=True,
                                stop=True,
                            )
                        kq = tpool.tile([P, t_half], BF16)
                        nc.vector.tensor_scalar(
                            kq[:],
                            u_ps[:],
                            MAGIC,
                            MAGIC,
                            mybir.AluOpType.add,
                            mybir.AluOpType.subtract,
                        )
                        for b in range(nb):
                            nc.tensor.matmul(
                                u_ps[:, b * 512 : (b + 1) * 512],
                                nident_bf[:],
                                kq[:, b * 512 : (b + 1) * 512],
                                start=False,
                                stop=True,
                                skip_group_check=True,
                            )
                        nc.scalar.activation(
                            out=s_sb[:, ci, :],
                            in_=u_ps[:],
                            func=mybir.ActivationFunctionType.Sin,
                            scale=SCALE_2PI,
                        )
                    nc.gpsimd.dma_start(
                        out=x_sb[:, 2 * cp : 2 * cp + 2, :],
                        in_=s_sb[:],
                        accum_op=mybir.AluOpType.add,
                    )
                    r0 = 2 * cp * P
                    nc.gpsimd.dma_start(
                        out=y_d[r0 : r0 + 2 * P, t0 : t0 + t_half].rearrange(
                            "(c p) t -> p c t", p=P
                        ),
                        in_=x_sb[:, 2 * cp : 2 * cp + 2, :],
                    )
                t0 += t_half

    nc.compile()
    return nc


_NC_CACHE: dict[tuple, object] = {}


def _get_nc(tok: int = ROWS, split: int = 2):
    key = (tok, split)
    nc = _NC_CACHE.get(key)
    if nc is None:
        nc = build_nc(tok, split)
        _NC_CACHE[key] = nc
    return nc


def _prep_weights(A_int8, B_int8, scale_A, scale_B):
    import ml_dtypes

    a2 = np.ascontiguousarray((A_int8.astype(np.float32) * 2.0).astype(np.float16))
    bpn = (
        scale_A.astype(np.float32)[:, None]
        * B_int8.astype(np.float32)
        * scale_B.astype(np.float32)[None, :]
        * np.float32(1.0 / (np.pi * np.pi))
    )
    bp_hi = bpn.astype(ml_dtypes.bfloat16)
    bp_lo = (bpn - bp_hi.astype(np.float32)).astype(ml_dtypes.bfloat16)
    # engines address partitions at 32-aligned starts only, so each 16-row
    # block sits at a 32-row offset; the zero pad rows kill the pad terms.
    z = np.zeros_like(bp_hi)
    bps = np.ascontiguousarray(
        np.concatenate([bp_hi, z, bp_hi, z, bp_lo, z], axis=0)
    )
    return a2, bps


def _prep_x_shard(xf, i, rows=ROWS):
    xs = xf[i * rows : (i + 1) * rows]  # [rows, 4096] f32
    return (xs.T * np.float32(0.5)).astype(np.float16)  # [4096, rows] C-contig


def kernel(x, A_int8, B_int8, scale_A, scale_B):
    x = np.asarray(x)
    orig_shape = x.shape
    xf = np.ascontiguousarray(x.reshape(TOTAL_ROWS, HIDDEN).astype(np.float32))
    a2, bps = _prep_weights(
        np.asarray(A_int8), np.asarray(B_int8), np.asarray(scale_A), np.asarray(scale_B)
    )

    nc = _get_nc(ROWS)
    in_maps = [
        {"xh": _prep_x_shard(xf, i), "A2": a2, "Bps": bps} for i in range(N_CORES)
    ]
    res = run_bass_kernel_spmd(nc, in_maps, core_ids=list(range(N_CORES)))
    y = np.concatenate(
        [r["out"].astype(np.float32).T for r in res.results], axis=0
    ) * np.float32(2.0)
    return np.ascontiguousarray(y.reshape(orig_shape)).astype(np.float32)


# revision 23
# speedup vs baseline: 1.1086x; 1.1086x over previous
"""PiLoraLayer TRN2 kernel: y = x + (alpha/r) * sin((2/pi) * (x @ A) @ B).

x: [4, 4096, 4096] f32; A = A_int8 * scale_A (per-col), B = B_int8 * scale_B
(per-col); rank 16 bottleneck.  alpha/r = 2.

Strategy v2 (data-parallel over 8 NeuronCores, TRANSPOSED compute space):

The v1 kernel was PE-bound: mm1 (h1 = x @ A) contracts over hidden, which
needs hidden on the partition axis, so every x tile went through a PE
transpose (512 fp32 transposes/core ~ 140us, plus 33% throttle).  v2 kills
all device transposes by shipping each core its token-shard PRE-TRANSPOSED
(hidden-major) from the host, and doing everything (mm1, mm2, sin, residual
add, output) in transposed space.

Precision plan (validated vs reference in numpy: rel err 6.1e-3 < 2e-2):
- Host ships xh = fp16(x.T / 2): fp16 halves DMA-in; /2 folds the final *2.
- mm1: h1[16, tok] = (2*A_int8 in fp16, exact).T @ xh  -> PSUM f32.
  (scale_A is folded into Bpn, so mm1 is exact except fp16(x).)
- mm2: fp32 matmuls run 4-pass on the PE (4x columns), so u is computed as
  ONE 1-pass bf16 matmul with the hi/lo split STACKED on the contraction
  axis (PE time ~ output columns, independent of K):
    u[128h, tok] = [h1_hi; h1_lo; h1_hi] (K=48) @ [Bp_hi; Bp_hi; Bp_lo]
  which is h1 @ Bpn to ~2^-17 relative.  Bpn = -scale_A x B x scale_B
  / pi^2, so 2*pi*u = -(2/pi)*h.  Bp hi/lo are split on the host; h1 hi/lo
  on device (2 ACT copies + 1 DVE subtract on a tiny [16, tok] tile).
- Range reduction (HW Sin LUT domain is [-pi, pi]; DVE `mod` fails the
  compiler ISA check, so): k = (u + 1.5*2^23) - 1.5*2^23 in one two-op DVE
  tensor_scalar (RNE round-to-int, bf16 exact for |k| <= 256); PE
  accumulates -k into the u PSUM bank via a bf16 negative-identity matmul,
  leaving frac in [-0.5, 0.5].
- ACT: s = fp16(Sin(frac * 2pi)) = -sin((2/pi) h).
- DVE: y_h = xh - s (all-fp16, 2x mode) = x/2 + sin((2/pi) h).
- DMA y_h out as fp16; host computes y = 2 * y_h.T.
  (Tried instead: SBUF->SBUF accum-DMA via gpsimd SWDGE CCE add — 19us
  SLOWER: the software-DGE accum path is low-bandwidth.  Keep DVE TT.)

Per-core budget (tokens=2048, hidden=4096): DMA 16 MiB in + 16 MiB out
(~91us floor at 358 GB/s); PE ~70-90us (small matmuls only); DVE 2 ops/elem
(~85us); ACT 1 op/elem (~60us).  Tokens are processed in 2 halves of 1024 so
the half-B input DMA overlaps the half-A tail.
"""

import sys

sys.path.insert(0, "/opt/trn_rl_repo")

import numpy as np

import concourse.bacc as bacc
import concourse.tile as tile
from concourse import mybir
from concourse.bass_utils import run_bass_kernel_spmd

P = 128
HIDDEN = 4096
RANK = 16
N_CORES = 8
TOTAL_ROWS = 4 * 4096
ROWS = TOTAL_ROWS // N_CORES  # 2048 tokens per core
KC = HIDDEN // P  # 32 hidden chunks
MAGIC = 12582912.0  # 1.5 * 2^23: f32 add/sub rounds to nearest integer
SCALE_2PI = 6.283185  # slightly < 2*pi: LUT arg stays inside [-pi, pi]

F32 = mybir.dt.float32
F32R = mybir.dt.float32r  # replicated fp32: 1 cycle/row on PE when N>=256
FP16 = mybir.dt.float16
BF16 = mybir.dt.bfloat16


def build_nc(tok: int = ROWS, split: int = 2):
    """Per-core program for a transposed [4096, tok] fp16 token shard."""
    halves = [tok // split] * split
    assert sum(halves) == tok and all(h % 512 == 0 for h in halves)

    nc = bacc.Bacc(
        "TRN2",
        target_bir_lowering=False,
        debug=False,
        enable_asserts=False,
        num_devices=N_CORES,
    )
    x_d = nc.dram_tensor("xh", [HIDDEN, tok], FP16, kind="ExternalInput").ap()
    a_d = nc.dram_tensor("A2", [HIDDEN, RANK], FP16, kind="ExternalInput").ap()
    bp_d = nc.dram_tensor("Bps", [6 * RANK, HIDDEN], BF16, kind="ExternalInput").ap()
    y_d = nc.dram_tensor("out", [HIDDEN, tok], FP16, kind="ExternalOutput").ap()

    with tile.TileContext(nc) as tc:
        with (
            tc.tile_pool(name="singles", bufs=1) as singles,
            tc.tile_pool(name="xp", bufs=2) as xpool,
            tc.tile_pool(name="h1p", bufs=1, space="PSUM") as h1_psum,
            tc.tile_pool(name="h1s", bufs=2) as h1pool,
            tc.tile_pool(name="up", bufs=3, space="PSUM") as u_psum,
            tc.tile_pool(name="tp", bufs=3) as tpool,
            tc.tile_pool(name="sp", bufs=3) as spool,
            tc.tile_pool(name="yp", bufs=3) as ypool,
        ):
            a_sb = singles.tile([P, KC, RANK], FP16)
            nc.sync.dma_start(
                out=a_sb[:], in_=a_d.rearrange("(k p) r -> p k r", p=P)
            )
            bp_sb = singles.tile([6 * RANK, HIDDEN], BF16)
            nc.sync.dma_start(out=bp_sb[:], in_=bp_d[:, :])
            nident_bf = singles.tile([P, P], BF16)
            nc.gpsimd.memset(nident_bf[:], 0.0)
            nc.gpsimd.affine_select(
                out=nident_bf[:],
                in_=nident_bf[:],
                compare_op=mybir.AluOpType.not_equal,
                fill=-1.0,
                base=0,
                pattern=[[-1, P]],
                channel_multiplier=1,
            )

            t0 = 0
            for t_half in halves:
                nb = t_half // 512  # psum-bank-wide blocks in this half
                # ---- head: DMA x k-chunks in, mm1 accumulate over k ----
                x_sb = xpool.tile([P, KC, t_half], FP16)
                for k in range(KC):
                    nc.sync.dma_start(
                        out=x_sb[:, k, :],
                        in_=x_d[k * P : (k + 1) * P, t0 : t0 + t_half],
                    )
                h1_ps = h1_psum.tile([RANK, t_half], F32)
                for k in range(KC):
                    for b in range(nb):
                        nc.tensor.matmul(
                            h1_ps[:, b * 512 : (b + 1) * 512],
                            a_sb[:, k, :],
                            x_sb[:, k, b * 512 : (b + 1) * 512],
                            start=(k == 0),
                            stop=(k == KC - 1),
                        )
                # h1 hi/lo bf16 split, stacked [h1_hi; _; h1_lo; _; h1_hi; _]
                # at 32-aligned starts, for the single K=96 bf16 mm2
                h1_sb = h1pool.tile([6 * RANK, t_half], BF16)
                nc.gpsimd.memset(h1_sb[:], 0.0)
                nc.scalar.copy(out=h1_sb[0:RANK, :], in_=h1_ps[:])
                nc.vector.tensor_tensor(
                    h1_sb[2 * RANK : 3 * RANK, :],
                    h1_ps[:],
                    h1_sb[0:RANK, :],
                    mybir.AluOpType.subtract,
                )
                nc.scalar.copy(out=h1_sb[4 * RANK : 5 * RANK, :], in_=h1_ps[:])

                # ---- tail: chunk PAIRS: mm2+round+sin per chunk, then one
                # [128, 2*t_half] subtract and one two-row-group DMA out ----
                for cp in range(KC // 2):
                    s_sb = spool.tile([P, 2, t_half], FP16)
                    for ci in range(2):
                        c = 2 * cp + ci
                        u_ps = u_psum.tile([P, t_half], F32)
                        for b in range(nb):
                            nc.tensor.matmul(
                                u_ps[:, b * 512 : (b + 1) * 512],
                                bp_sb[:, c * P : (c + 1) * P],
                                h1_sb[:, b * 512 : (b + 1) * 512],
                                start=True,
                                stop=True,
                            )
                        kq = tpool.tile([P, t_half], BF16)
                        nc.vector.tensor_scalar(
                            kq[:],
                            u_ps[:],
                            MAGIC,
                            MAGIC,
                            mybir.AluOpType.add,
                            mybir.AluOpType.subtract,
                        )
                        for b in range(nb):
                            nc.tensor.matmul(
                                u_ps[:, b * 512 : (b + 1) * 512],
                                nident_bf[:],
                                kq[:, b * 512 : (b + 1) * 512],
                                start=False,
                                stop=True,
                                skip_group_check=True,
                            )
                        nc.scalar.activation(
                            out=s_sb[:, ci, :],
                            in_=u_ps[:],
                            func=mybir.ActivationFunctionType.Sin,
                            scale=SCALE_2PI,
                        )
                    y_sb = ypool.tile([P, 2, t_half], FP16)
                    nc.vector.tensor_tensor(
                        y_sb[:],
                        x_sb[:, 2 * cp : 2 * cp + 2, :],
                        s_sb[:],
                        mybir.AluOpType.subtract,
                    )
                    r0 = 2 * cp * P
                    nc.gpsimd.dma_start(
                        out=y_d[r0 : r0 + 2 * P, t0 : t0 + t_half].rearrange(
                            "(c p) t -> p c t", p=P
                        ),
                        in_=y_sb[:],
                    )
                t0 += t_half

    nc.compile()
    return nc


_NC_CACHE: dict[tuple, object] = {}


def _get_nc(tok: int = ROWS, split: int = 2):
    key = (tok, split)
    nc = _NC_CACHE.get(key)
    if nc is None:
        nc = build_nc(tok, split)
        _NC_CACHE[key] = nc
    return nc


def _prep_weights(A_int8, B_int8, scale_A, scale_B):
    import ml_dtypes

    a2 = np.ascontiguousarray((A_int8.astype(np.float32) * 2.0).astype(np.float16))
    bpn = (
        -scale_A.astype(np.float32)[:, None]
        * B_int8.astype(np.float32)
        * scale_B.astype(np.float32)[None, :]
        * np.float32(1.0 / (np.pi * np.pi))
    )
    bp_hi = bpn.astype(ml_dtypes.bfloat16)
    bp_lo = (bpn - bp_hi.astype(np.float32)).astype(ml_dtypes.bfloat16)
    # engines address partitions at 32-aligned starts only, so each 16-row
    # block sits at a 32-row offset; the zero pad rows kill the pad terms.
    z = np.zeros_like(bp_hi)
    bps = np.ascontiguousarray(
        np.concatenate([bp_hi, z, bp_hi, z, bp_lo, z], axis=0)
    )
    return a2, bps


def _prep_x_shard(xf, i, rows=ROWS):
    xs = xf[i * rows : (i + 1) * rows]  # [rows, 4096] f32
    return (xs.T * np.float32(0.5)).astype(np.float16)  # [4096, rows] C-contig


def kernel(x, A_int8, B_int8, scale_A, scale_B):
    x = np.asarray(x)
    orig_shape = x.shape
    xf = np.ascontiguousarray(x.reshape(TOTAL_ROWS, HIDDEN).astype(np.float32))
    a2, bps = _prep_weights(
        np.asarray(A_int8), np.asarray(B_int8), np.asarray(scale_A), np.asarray(scale_B)
    )

    nc = _get_nc(ROWS)
    in_maps = [
        {"xh": _prep_x_shard(xf, i), "A2": a2, "Bps": bps} for i in range(N_CORES)
    ]
    res = run_bass_kernel_spmd(nc, in_maps, core_ids=list(range(N_CORES)))
    y = np.concatenate(
        [r["out"].astype(np.float32).T for r in res.results], axis=0
    ) * np.float32(2.0)
    return np.ascontiguousarray(y.reshape(orig_shape)).astype(np.float32)
